# revision 1
# baseline (speedup 1.0000x reference)
"""FFF (fast feedforward / MoE tree-routing) Trainium2 kernel.

Strategy (8 NeuronCores, SPMD):
  Launch 1 — routing, data-parallel over batch: each core routes 1024 samples
    through the depth-11 plane tree. Levels 0..6 are evaluated densely
    (scores for all 127 shallow nodes via fp32 matmuls, per-sample select via
    iota/is_equal mask), levels 7..10 gather the [w|b] node rows with a bulk
    SWDGE dma_gather and reduce on VectorE.
  Host — slot assignment: samples grouped by leaf expert; leaves sharded
    expert-parallel 256/core, 8 experts per group, fixed 96-slot capacity per
    group; x rows gathered+transposed on the host into each core's input.
  Launch 2 — expert MLP, expert-parallel: per 8-expert group one fused
    [768x128] @ [768xS] fp32 matmul chain computes all 8 experts' h lanes,
    bias+relu+lane-mask on VectorE, then h.T @ W2stack produces the output
    rows. Weights stream through SBUF once per core (25 MB).
  Host — scatter output rows back to sample order.
"""

import contextlib
import numpy as np

import concourse.bacc as bacc
import concourse.mybir as mybir
import concourse.tile as tile
from concourse.bass import ts
from concourse.mybir import AluOpType, AxisListType
from concourse.bass_utils import run_bass_kernel_spmd

# problem shapes (hardcoded per contract)
DEPTH = 11
IN_W = 768
LEAF_W = 16
OUT_W = 768
N_NODES = 2047
N_LEAVES = 2048
BATCH = 8192
N_CORES = 8

# routing kernel layout
B_CORE = BATCH // N_CORES            # 1024
EXT = 832                            # gather row [w(768) | b | pad] (3328B, %256)
KD = 896                             # 7*128 dense-contraction rows [x | 1 | pad]
DENSE_LEVELS = 7                     # levels 0..6 dense (127 nodes)

# mlp kernel layout
LEAVES_PER_CORE = N_LEAVES // N_CORES       # 256
EXPERTS_PER_GROUP = 8
GROUPS = LEAVES_PER_CORE // EXPERTS_PER_GROUP   # 32
SLOTS_PER_GROUP = 96
SLOTS = GROUPS * SLOTS_PER_GROUP            # 3072
KC = IN_W // 128                            # 6

F32 = mybir.dt.float32
I32 = mybir.dt.int32
I16 = mybir.dt.int16


# ---------------------------------------------------------------- launch 1
def _build_routing_nc():
    nc = bacc.Bacc("TRN2", target_bir_lowering=False, debug=False,
                   num_devices=N_CORES)
    xT = nc.dram_tensor("xT", [KD, B_CORE], F32, kind="ExternalInput").ap()
    xe = nc.dram_tensor("xe", [B_CORE, EXT], F32, kind="ExternalInput").ap()
    wd = nc.dram_tensor("wd", [KD, 128], F32, kind="ExternalInput").ap()
    nwe = nc.dram_tensor("nwe", [N_LEAVES, EXT], F32, kind="ExternalInput").ap()
    leaf = nc.dram_tensor("leaf", [B_CORE], I32, kind="ExternalOutput").ap()
    idxs_dram = nc.dram_tensor("idxs_scratch", [B_CORE], I16, kind="Internal").ap()

    with tile.TileContext(nc) as tc, contextlib.ExitStack() as ctx:
        pool = ctx.enter_context(tc.tile_pool(name="sbuf", bufs=1))
        wpool = ctx.enter_context(tc.tile_pool(name="work", bufs=2))
        psum = ctx.enter_context(tc.tile_pool(name="psum", bufs=2, space="PSUM"))

        xT_sb = pool.tile([128, 7, B_CORE], F32)
        xe_sb = pool.tile([128, 8, EXT], F32)
        wd_sb = pool.tile([128, 7, 128], F32)
        nc.sync.dma_start(out=xT_sb[:], in_=xT.rearrange("(k p) s -> p k s", p=128))
        nc.sync.dma_start(out=xe_sb[:], in_=xe.rearrange("(c p) d -> p c d", p=128))
        nc.sync.dma_start(out=wd_sb[:], in_=wd.rearrange("(k p) n -> p k n", p=128))

        # dense scores S[p, c, n] = x . w_n + b_n for nodes n in [0, 127)
        s_sb = pool.tile([128, 8, 128], F32)
        for c in range(8):
            ps = psum.tile([128, 128], F32, space="PSUM")
            for k in range(7):
                nc.tensor.matmul(
                    ps[:], lhsT=xT_sb[:, k, ts(c, 128)], rhs=wd_sb[:, k, :],
                    start=(k == 0), stop=(k == 6),
                )
            nc.vector.tensor_copy(out=s_sb[:, c, :], in_=ps[:])

        iota_i = pool.tile([128, 8, 64], I32)
        iota_f = pool.tile([128, 8, 64], F32)
        nc.gpsimd.iota(iota_i[:], pattern=[[0, 8], [1, 64]], base=0,
                       channel_multiplier=0)
        nc.vector.tensor_copy(out=iota_f[:], in_=iota_i[:])

        cur = pool.tile([128, 8], F32)
        choice = pool.tile([128, 8], F32)
        sel = pool.tile([128, 8], F32)
        rtile = pool.tile([128, 8], F32)

        nc.vector.tensor_scalar(out=choice[:], in0=s_sb[:, :, 0], scalar1=0.0,
                                scalar2=None, op0=AluOpType.is_ge)
        nc.vector.tensor_scalar_add(out=cur[:], in0=choice[:], scalar1=1.0)

        mask = pool.tile([128, 8, 64], F32)
        prod = pool.tile([128, 8, 64], F32)
        for lvl in range(1, DENSE_LEVELS):
            n = 2 ** lvl
            off = n - 1
            nc.vector.tensor_scalar_sub(out=rtile[:], in0=cur[:], scalar1=float(off))
            nc.vector.tensor_tensor(
                out=mask[:, :, :n], in0=iota_f[:, :, :n],
                in1=rtile[:, :, None].to_broadcast([128, 8, n]),
                op=AluOpType.is_equal,
            )
            nc.vector.tensor_tensor(
                out=prod[:, :, :n], in0=mask[:, :, :n],
                in1=s_sb[:, :, off:off + n], op=AluOpType.mult,
            )
            nc.vector.tensor_reduce(out=sel[:], in_=prod[:, :, :n],
                                    axis=AxisListType.X, op=AluOpType.add)
            nc.vector.tensor_scalar(out=choice[:], in0=sel[:], scalar1=0.0,
                                    scalar2=None, op0=AluOpType.is_ge)
            nc.vector.tensor_scalar_mul(out=cur[:], in0=cur[:], scalar1=2.0)
            nc.vector.tensor_add(out=cur[:], in0=cur[:], in1=choice[:])
            nc.vector.tensor_scalar_add(out=cur[:], in0=cur[:], scalar1=1.0)

        for lvl in range(DENSE_LEVELS, DEPTH):
            cur16 = wpool.tile([128, 8], I16)
            nc.vector.tensor_copy(out=cur16[:], in_=cur[:])
            nc.sync.dma_start(out=idxs_dram.rearrange("(c p) -> p c", p=128),
                              in_=cur16[:])
            idx_sb = wpool.tile([128, 64], I16)
            ap16 = idxs_dram.rearrange("(q ch) -> ch q", ch=16)
            for r in range(8):
                nc.sync.dma_start(out=idx_sb[ts(r, 16), :], in_=ap16)
            gath = wpool.tile([128, 8, EXT], F32)
            nc.gpsimd.dma_gather(
                out_ap=gath[:], in_ap=nwe[:], idxs_ap=idx_sb[:],
                num_idxs=B_CORE, num_idxs_reg=B_CORE, elem_size=EXT,
            )
            prodg = wpool.tile([128, 8, EXT], F32)
            nc.vector.tensor_tensor(out=prodg[:], in0=xe_sb[:], in1=gath[:],
                                    op=AluOpType.mult)
            nc.vector.tensor_reduce(out=sel[:], in_=prodg[:],
                                    axis=AxisListType.X, op=AluOpType.add)
            nc.vector.tensor_scalar(out=choice[:], in0=sel[:], scalar1=0.0,
                                    scalar2=None, op0=AluOpType.is_ge)
            nc.vector.tensor_scalar_mul(out=cur[:], in0=cur[:], scalar1=2.0)
            nc.vector.tensor_add(out=cur[:], in0=cur[:], in1=choice[:])
            nc.vector.tensor_scalar_add(out=cur[:], in0=cur[:], scalar1=1.0)

        nc.vector.tensor_scalar_sub(out=cur[:], in0=cur[:], scalar1=float(N_NODES))
        leaf_i = pool.tile([128, 8], I32)
        nc.vector.tensor_copy(out=leaf_i[:], in_=cur[:])
        nc.sync.dma_start(out=leaf.rearrange("(c p) -> p c", p=128), in_=leaf_i[:])

    nc.compile()
    return nc


def _host_prep_routing(x, node_weights, node_biases):
    """Per-core routing inputs. Shared tensors are passed to every core."""
    n_dense = 2 ** DENSE_LEVELS - 1
    wd = np.zeros((KD, 128), np.float32)
    wd[:IN_W, :n_dense] = node_weights[:n_dense].T
    wd[IN_W, :n_dense] = node_biases[:n_dense]
    nwe = np.zeros((N_LEAVES, EXT), np.float32)
    nwe[:N_NODES, :IN_W] = node_weights
    nwe[:N_NODES, IN_W] = node_biases

    in_maps = []
    for c in range(N_CORES):
        xs = x[c * B_CORE:(c + 1) * B_CORE]
        xT = np.zeros((KD, B_CORE), np.float32)
        xT[:IN_W] = xs.T
        xT[IN_W] = 1.0
        xe = np.zeros((B_CORE, EXT), np.float32)
        xe[:, :IN_W] = xs
        xe[:, IN_W] = 1.0
        in_maps.append({"xT": xT, "xe": xe, "wd": wd, "nwe": nwe})
    return in_maps


# ---------------------------------------------------------------- launch 2
def _build_mlp_nc():
    nc = bacc.Bacc("TRN2", target_bir_lowering=False, debug=False,
                   num_devices=N_CORES)
    xgT = nc.dram_tensor("xgT", [IN_W, SLOTS], F32, kind="ExternalInput").ap()
    w1f = nc.dram_tensor("w1f", [GROUPS, KC, 128, 128], F32,
                         kind="ExternalInput").ap()
    w2f = nc.dram_tensor("w2f", [GROUPS, 128, OUT_W], F32,
                         kind="ExternalInput").ap()
    b1bc = nc.dram_tensor("b1bc", [128, SLOTS], F32, kind="ExternalInput").ap()
    maskt = nc.dram_tensor("maskt", [128, SLOTS], F32, kind="ExternalInput").ap()
    out = nc.dram_tensor("o", [SLOTS, OUT_W], F32, kind="ExternalOutput").ap()

    with tile.TileContext(nc) as tc, contextlib.ExitStack() as ctx:
        pool = ctx.enter_context(tc.tile_pool(name="sbuf", bufs=1))
        wpool = ctx.enter_context(tc.tile_pool(name="w", bufs=3))
        hpool = ctx.enter_context(tc.tile_pool(name="h", bufs=3))
        ps1 = ctx.enter_context(tc.tile_pool(name="ps1", bufs=2, space="PSUM"))
        ps2 = ctx.enter_context(tc.tile_pool(name="ps2", bufs=2, space="PSUM"))

        xt_sb = pool.tile([128, KC, SLOTS], F32)
        nc.sync.dma_start(out=xt_sb[:], in_=xgT.rearrange("(k p) s -> p k s", p=128))
        b1_sb = pool.tile([128, SLOTS], F32)
        nc.sync.dma_start(out=b1_sb[:], in_=b1bc[:])
        mask_sb = pool.tile([128, SLOTS], F32)
        nc.sync.dma_start(out=mask_sb[:], in_=maskt[:])

        for g in range(GROUPS):
            w1_sb = wpool.tile([128, KC, 128], F32, tag="w1")
            w2_sb = wpool.tile([128, OUT_W], F32, tag="w2")
            nc.sync.dma_start(out=w1_sb[:], in_=w1f[g].rearrange("k p n -> p k n"))
            nc.sync.dma_start(out=w2_sb[:], in_=w2f[g])

            sl = ts(g, SLOTS_PER_GROUP)
            p1 = ps1.tile([128, SLOTS_PER_GROUP], F32, space="PSUM")
            for k in range(KC):
                nc.tensor.matmul(
                    p1[:], lhsT=w1_sb[:, k, :], rhs=xt_sb[:, k, sl],
                    start=(k == 0), stop=(k == KC - 1),
                )

            hf = hpool.tile([128, SLOTS_PER_GROUP], F32, tag="hf")
            nc.vector.tensor_add(out=hf[:], in0=p1[:], in1=b1_sb[:, sl])
            nc.vector.tensor_scalar_max(out=hf[:], in0=hf[:], scalar1=0.0)
            nc.vector.tensor_mul(out=hf[:], in0=hf[:], in1=mask_sb[:, sl])

            NH = OUT_W // 2
            p2a = ps2.tile([SLOTS_PER_GROUP, NH], F32, space="PSUM", tag="p2a")
            p2b = ps2.tile([SLOTS_PER_GROUP, NH], F32, space="PSUM", tag="p2b")
            nc.tensor.matmul(p2a[:], lhsT=hf[:], rhs=w2_sb[:, :NH],
                             start=True, stop=True)
            nc.tensor.matmul(p2b[:], lhsT=hf[:], rhs=w2_sb[:, NH:],
                             start=True, stop=True)
            o_sb = hpool.tile([SLOTS_PER_GROUP, OUT_W], F32, tag="o")
            nc.vector.tensor_copy(out=o_sb[:, :NH], in_=p2a[:])
            nc.vector.tensor_copy(out=o_sb[:, NH:], in_=p2b[:])
            nc.sync.dma_start(out=out[sl, :], in_=o_sb[:])

    nc.compile()
    return nc


def _host_prep_mlp(leaves, x, w1s, b1s, w2s):
    """Per-core MLP inputs + slot->sample maps."""
    in_maps, slot_maps = [], []
    order = np.argsort(leaves, kind="stable")
    sorted_leaves = leaves[order]
    for c in range(N_CORES):
        lo, hi = LEAVES_PER_CORE * c, LEAVES_PER_CORE * (c + 1)
        beg, end = np.searchsorted(sorted_leaves, [lo, hi])
        samples = order[beg:end]
        l_loc = leaves[samples] - lo
        g_all = l_loc // EXPERTS_PER_GROUP
        e_all = l_loc % EXPERTS_PER_GROUP
        # rank within group (samples sorted by leaf -> sorted by group)
        slot = np.empty(len(samples), np.int64)
        fill = np.zeros(GROUPS, np.int64)
        for i, g in enumerate(g_all):
            slot[i] = SLOTS_PER_GROUP * g + fill[g]
            fill[g] += 1
        if fill.max() > SLOTS_PER_GROUP:
            raise RuntimeError(f"group capacity exceeded: {fill.max()}")

        slot_sample = np.full(SLOTS, -1, np.int64)
        slot_sample[slot] = samples
        mask = np.zeros((128, SLOTS), np.float32)
        lane_rows = (16 * e_all[None, :] + np.arange(16)[:, None])  # [16, n]
        mask[lane_rows, slot[None, :]] = 1.0

        xg = np.zeros((SLOTS, IN_W), np.float32)
        xg[slot] = x[samples]
        xgT = np.ascontiguousarray(xg.T)

        w1c = w1s[lo:hi]
        w1f = np.ascontiguousarray(
            w1c.reshape(GROUPS, 8, IN_W, LEAF_W)
            .transpose(0, 2, 1, 3)
            .reshape(GROUPS, IN_W, 128)
            .reshape(GROUPS, KC, 128, 128)
        )
        w2f = np.ascontiguousarray(w2s[lo:hi].reshape(GROUPS, 128, OUT_W))
        lanes = b1s[lo:hi].reshape(GROUPS, 128)
        b1bc = np.repeat(lanes[:, :, None], SLOTS_PER_GROUP, axis=2)
        b1bc = np.ascontiguousarray(
            b1bc.transpose(1, 0, 2).reshape(128, SLOTS)
        ).astype(np.float32)

        in_maps.append({"xgT": xgT, "w1f": w1f, "w2f": w2f,
                        "b1bc": b1bc, "maskt": mask})
        slot_maps.append(slot_sample)
    return in_maps, slot_maps


# ---------------------------------------------------------------- entry
def kernel(x, node_weights, node_biases, w1s, b1s, w2s):
    x = np.ascontiguousarray(np.asarray(x, np.float32))
    node_weights = np.ascontiguousarray(np.asarray(node_weights, np.float32))
    node_biases = np.ascontiguousarray(np.asarray(node_biases, np.float32))
    w1s = np.asarray(w1s, np.float32)
    b1s = np.asarray(b1s, np.float32)
    w2s = np.asarray(w2s, np.float32)

    # launch 1: routing
    nc1 = _build_routing_nc()
    in1 = _host_prep_routing(x, node_weights, node_biases)
    res1 = run_bass_kernel_spmd(nc1, in1, core_ids=list(range(N_CORES)))
    leaves = np.concatenate([res1.results[c]["leaf"] for c in range(N_CORES)])
    leaves = leaves.astype(np.int64)

    # launch 2: expert MLP
    nc2 = _build_mlp_nc()
    in2, slot_maps = _host_prep_mlp(leaves, x, w1s, b1s, w2s)
    res2 = run_bass_kernel_spmd(nc2, in2, core_ids=list(range(N_CORES)))

    out = np.zeros((BATCH, OUT_W), np.float32)
    for c in range(N_CORES):
        o_slots = res2.results[c]["o"]
        sm = slot_maps[c]
        valid = sm >= 0
        out[sm[valid]] = o_slots[valid]
    return out


# revision 2
# speedup vs baseline: 1.1756x; 1.1756x over previous
"""FFF (fast feedforward / MoE tree-routing) Trainium2 kernel.

Strategy (8 NeuronCores, SPMD, two launches):
  Launch 1 — routing, data-parallel over batch: each core routes 1024 samples
    through the depth-11 plane tree. Levels 0..7 are evaluated densely
    (scores for all 255 shallow nodes via fp32 matmuls against host-packed
    [x|1] / [w|b] operands, per-sample select via iota/is_equal mask).
    Levels 8..10 gather each sample's [w|b] node row with bulk SWDGE
    dma_gathers and reduce on VectorE; four independent quarter-pipelines
    overlap the gather DMA chains with the other quarters' dots.
  Host — slot assignment: samples grouped by leaf expert; leaves sharded
    expert-parallel 256/core, 8 experts per group, fixed 80-slot capacity per
    group; x rows gathered+transposed on the host into each core's input.
  Launch 2 — expert MLP, expert-parallel: per 8-expert group one fused
    [768x128] @ [768x80] fp32 matmul chain computes all 8 experts' h lanes
    at once, bias+relu+lane-mask on VectorE, then h.T @ W2stack produces the
    output rows. Weights stream through SBUF once per core (25 MB).
  Host — scatter output rows back to sample order.
"""

import contextlib
import numpy as np

import concourse.bacc as bacc
import concourse.mybir as mybir
import concourse.tile as tile
from concourse.bass import ts
from concourse.mybir import AluOpType, AxisListType
from concourse.bass_utils import run_bass_kernel_spmd

# problem shapes (hardcoded per contract)
DEPTH = 11
IN_W = 768
LEAF_W = 16
OUT_W = 768
N_NODES = 2047
N_LEAVES = 2048
BATCH = 8192
N_CORES = 8

# routing kernel layout
B_CORE = BATCH // N_CORES            # 1024
EXT = 832                            # gather row [w(768) | b | pad] (3328B, %256)
DOT = IN_W + 1                       # useful columns of a gathered row
KD = 896                             # 7*128 dense-contraction rows [x | 1 | pad]
DENSE_LEVELS = 8                     # levels 0..7 dense (255 nodes)
N_DENSE = 2 ** DENSE_LEVELS - 1      # 255
NQ = 4                               # routing gather pipelines
CQ = 8 // NQ                         # c-tiles per quarter
QN = B_CORE // NQ                    # samples per quarter

# mlp kernel layout
LEAVES_PER_CORE = N_LEAVES // N_CORES           # 256
EXPERTS_PER_GROUP = 8
GROUPS = LEAVES_PER_CORE // EXPERTS_PER_GROUP   # 32
SLOTS_PER_GROUP = 80
SLOTS = GROUPS * SLOTS_PER_GROUP                # 2560
KC = IN_W // 128                                # 6

F32 = mybir.dt.float32
I32 = mybir.dt.int32
I16 = mybir.dt.int16


# ---------------------------------------------------------------- launch 1
def _build_routing_nc():
    nc = bacc.Bacc("TRN2", target_bir_lowering=False, debug=False,
                   num_devices=N_CORES)
    xT = nc.dram_tensor("xT", [KD, B_CORE], F32, kind="ExternalInput").ap()
    xe = nc.dram_tensor("xe", [B_CORE, EXT], F32, kind="ExternalInput").ap()
    wd = nc.dram_tensor("wd", [KD, 256], F32, kind="ExternalInput").ap()
    nwe = nc.dram_tensor("nwe", [N_LEAVES, EXT], F32, kind="ExternalInput").ap()
    leaf = nc.dram_tensor("leaf", [B_CORE], I32, kind="ExternalOutput").ap()
    idxs_dram = [
        nc.dram_tensor(f"idxs_scratch{q}", [QN], I16, kind="Internal").ap()
        for q in range(NQ)
    ]

    with tile.TileContext(nc) as tc, contextlib.ExitStack() as ctx:
        pool = ctx.enter_context(tc.tile_pool(name="sbuf", bufs=1))
        wpool = ctx.enter_context(tc.tile_pool(name="work", bufs=2))
        psum = ctx.enter_context(tc.tile_pool(name="psum", bufs=2, space="PSUM"))

        xT_sb = pool.tile([128, 7, B_CORE], F32)
        xe_sb = pool.tile([128, 8, EXT], F32)
        wd_sb = pool.tile([128, 7, 256], F32)
        xT_r = xT.rearrange("(k p) s -> p k s", p=128)
        for k in range(7):
            nc.sync.dma_start(out=xT_sb[:, k, :], in_=xT_r[:, k, :])
        nc.sync.dma_start(out=xe_sb[:], in_=xe.rearrange("(c p) d -> p c d", p=128))
        nc.sync.dma_start(out=wd_sb[:], in_=wd.rearrange("(k p) n -> p k n", p=128))

        # dense scores S[p, c, n] = x . w_n + b_n for nodes n in [0, 255)
        s_sb = pool.tile([128, 8, 256], F32)
        for c in range(8):
            ps = psum.tile([128, 256], F32, space="PSUM")
            for k in range(7):
                nc.tensor.matmul(
                    ps[:], lhsT=xT_sb[:, k, ts(c, 128)], rhs=wd_sb[:, k, :],
                    start=(k == 0), stop=(k == 6),
                )
            nc.vector.tensor_copy(out=s_sb[:, c, :], in_=ps[:])

        # iota of global node index so mask = is_equal(iota[off:off+n], cur)
        iota_i = pool.tile([128, 8, 256], I32)
        iota_f = pool.tile([128, 8, 256], F32)
        nc.gpsimd.iota(iota_i[:], pattern=[[0, 8], [1, 256]], base=0,
                       channel_multiplier=0)
        nc.vector.tensor_copy(out=iota_f[:], in_=iota_i[:])

        cur = pool.tile([128, 8], F32)
        choice = pool.tile([128, 8], F32)
        sel = pool.tile([128, 8], F32)
        tmp2 = pool.tile([128, 8], F32)

        nc.vector.tensor_scalar(out=choice[:], in0=s_sb[:, :, 0], scalar1=0.0,
                                scalar2=None, op0=AluOpType.is_ge)
        nc.vector.tensor_scalar_add(out=cur[:], in0=choice[:], scalar1=1.0)

        mask = pool.tile([128, 8, 128], F32)
        prod = pool.tile([128, 8, 128], F32)
        for lvl in range(1, DENSE_LEVELS):
            n = 2 ** lvl
            off = n - 1
            nc.vector.tensor_tensor(
                out=mask[:, :, :n], in0=iota_f[:, :, off:off + n],
                in1=cur[:, :, None].to_broadcast([128, 8, n]),
                op=AluOpType.is_equal,
            )
            nc.vector.tensor_tensor(
                out=prod[:, :, :n], in0=mask[:, :, :n],
                in1=s_sb[:, :, off:off + n], op=AluOpType.mult,
            )
            nc.vector.tensor_reduce(out=sel[:], in_=prod[:, :, :n],
                                    axis=AxisListType.X, op=AluOpType.add)
            nc.vector.tensor_scalar(out=choice[:], in0=sel[:], scalar1=0.0,
                                    scalar2=None, op0=AluOpType.is_ge)
            nc.vector.tensor_scalar(out=tmp2[:], in0=cur[:], scalar1=2.0,
                                    scalar2=1.0, op0=AluOpType.mult,
                                    op1=AluOpType.add)
            nc.vector.tensor_add(out=cur[:], in0=tmp2[:], in1=choice[:])

        # gather levels 8..10: NQ independent quarter-pipelines
        quarters = [(q, slice(CQ * q, CQ * (q + 1))) for q in range(NQ)]
        cur_q, sel_q, ch_q = {}, {}, {}
        for q, csl in quarters:
            cur_q[q] = pool.tile([128, CQ], F32, tag=f"cur{q}", name=f"cur{q}")
            sel_q[q] = pool.tile([128, CQ], F32, tag=f"sel{q}", name=f"sel{q}")
            ch_q[q] = pool.tile([128, CQ], F32, tag=f"ch{q}", name=f"chq{q}")
            nc.vector.tensor_copy(out=cur_q[q][:], in_=cur[:, csl])

        def issue_gather(q):
            cv = cur_q[q]
            cur16 = wpool.tile([128, CQ], I16, tag=f"c16{q}", name=f"c16{q}")
            nc.vector.tensor_copy(out=cur16[:], in_=cv[:])
            nc.sync.dma_start(
                out=idxs_dram[q].rearrange("(c p) -> p c", p=128), in_=cur16[:]
            )
            idx_sb = wpool.tile([128, QN // 16], I16, tag=f"idx{q}", name=f"idx{q}")
            ap16 = idxs_dram[q].rearrange("(s ch) -> ch s", ch=16)
            for r in range(8):
                nc.sync.dma_start(out=idx_sb[ts(r, 16), :], in_=ap16)
            gath = wpool.tile([128, CQ, EXT], F32, tag=f"g{q}", name=f"g{q}")
            nc.gpsimd.dma_gather(
                out_ap=gath[:], in_ap=nwe[:], idxs_ap=idx_sb[:],
                num_idxs=QN, num_idxs_reg=QN, elem_size=EXT,
            )
            return gath

        gaths = {q: issue_gather(q) for q, _ in quarters}
        for lvl in range(DENSE_LEVELS, DEPTH):
            next_g = {}
            for q, csl in quarters:
                cv, sv, chv = cur_q[q], sel_q[q], ch_q[q]
                prodg = wpool.tile([128, CQ, DOT], F32, tag=f"p{q}",
                                   name=f"p{q}", bufs=1)
                nc.vector.tensor_tensor(
                    out=prodg[:], in0=xe_sb[:, csl, :DOT],
                    in1=gaths[q][:, :, :DOT], op=AluOpType.mult,
                )
                nc.vector.tensor_reduce(out=sv[:], in_=prodg[:],
                                        axis=AxisListType.X, op=AluOpType.add)
                nc.vector.tensor_scalar(out=chv[:], in0=sv[:], scalar1=0.0,
                                        scalar2=None, op0=AluOpType.is_ge)
                nc.vector.tensor_scalar_mul(out=cv[:], in0=cv[:], scalar1=2.0)
                nc.vector.tensor_add(out=cv[:], in0=cv[:], in1=chv[:])
                nc.vector.tensor_scalar_add(out=cv[:], in0=cv[:], scalar1=1.0)
                if lvl + 1 < DEPTH:
                    next_g[q] = issue_gather(q)
            gaths = next_g

        leaf_i = pool.tile([128, 8], I32)
        for q, csl in quarters:
            nc.vector.tensor_scalar_sub(out=cur_q[q][:], in0=cur_q[q][:],
                                        scalar1=float(N_NODES))
            nc.vector.tensor_copy(out=leaf_i[:, csl], in_=cur_q[q][:])
        nc.sync.dma_start(out=leaf.rearrange("(c p) -> p c", p=128), in_=leaf_i[:])

    nc.compile()
    return nc


def _host_prep_routing(x, node_weights, node_biases):
    wd = np.zeros((KD, 256), np.float32)
    wd[:IN_W, :N_DENSE] = node_weights[:N_DENSE].T
    wd[IN_W, :N_DENSE] = node_biases[:N_DENSE]
    nwe = np.zeros((N_LEAVES, EXT), np.float32)
    nwe[:N_NODES, :IN_W] = node_weights
    nwe[:N_NODES, IN_W] = node_biases

    in_maps = []
    for c in range(N_CORES):
        xs = x[c * B_CORE:(c + 1) * B_CORE]
        xT = np.zeros((KD, B_CORE), np.float32)
        xT[:IN_W] = xs.T
        xT[IN_W] = 1.0
        xe = np.zeros((B_CORE, EXT), np.float32)
        xe[:, :IN_W] = xs
        xe[:, IN_W] = 1.0
        in_maps.append({"xT": xT, "xe": xe, "wd": wd, "nwe": nwe})
    return in_maps


# ---------------------------------------------------------------- launch 2
def _build_mlp_nc():
    nc = bacc.Bacc("TRN2", target_bir_lowering=False, debug=False,
                   num_devices=N_CORES)
    xgT = nc.dram_tensor("xgT", [IN_W, SLOTS], F32, kind="ExternalInput").ap()
    w1f = nc.dram_tensor("w1f", [GROUPS, 128, KC * 128], F32,
                         kind="ExternalInput").ap()
    w2f = nc.dram_tensor("w2f", [GROUPS, 128, OUT_W], F32,
                         kind="ExternalInput").ap()
    b1bc = nc.dram_tensor("b1bc", [128, GROUPS], F32, kind="ExternalInput").ap()
    maskt = nc.dram_tensor("maskt", [128, SLOTS], F32, kind="ExternalInput").ap()
    out = nc.dram_tensor("o", [SLOTS, OUT_W], F32, kind="ExternalOutput").ap()

    with tile.TileContext(nc) as tc, contextlib.ExitStack() as ctx:
        pool = ctx.enter_context(tc.tile_pool(name="sbuf", bufs=1))
        wpool = ctx.enter_context(tc.tile_pool(name="w", bufs=3))
        hpool = ctx.enter_context(tc.tile_pool(name="h", bufs=3))
        ps1 = ctx.enter_context(tc.tile_pool(name="ps1", bufs=2, space="PSUM"))
        ps2 = ctx.enter_context(tc.tile_pool(name="ps2", bufs=2, space="PSUM"))

        xt_sb = pool.tile([128, KC, SLOTS], F32)
        xt_r = xgT.rearrange("(k p) s -> p k s", p=128)
        for k in range(KC):
            nc.sync.dma_start(out=xt_sb[:, k, :], in_=xt_r[:, k, :])
        b1_sb = pool.tile([128, GROUPS], F32)
        nc.sync.dma_start(out=b1_sb[:], in_=b1bc[:])
        mask_sb = pool.tile([128, SLOTS], F32)
        nc.sync.dma_start(out=mask_sb[:], in_=maskt[:])

        for g in range(GROUPS):
            w1_sb = wpool.tile([128, KC, 128], F32, tag="w1")
            w2_sb = wpool.tile([128, OUT_W], F32, tag="w2")
            nc.sync.dma_start(out=w1_sb[:], in_=w1f[g])
            nc.sync.dma_start(out=w2_sb[:], in_=w2f[g])

            sl = ts(g, SLOTS_PER_GROUP)
            p1 = ps1.tile([128, SLOTS_PER_GROUP], F32, space="PSUM")
            for k in range(KC):
                nc.tensor.matmul(
                    p1[:], lhsT=w1_sb[:, k, :], rhs=xt_sb[:, k, sl],
                    start=(k == 0), stop=(k == KC - 1),
                )

            hf = hpool.tile([128, SLOTS_PER_GROUP], F32, tag="hf")
            nc.vector.tensor_add(
                out=hf[:], in0=p1[:],
                in1=b1_sb[:, g:g + 1].to_broadcast([128, SLOTS_PER_GROUP]),
            )
            nc.vector.tensor_scalar_max(out=hf[:], in0=hf[:], scalar1=0.0)
            nc.vector.tensor_mul(out=hf[:], in0=hf[:], in1=mask_sb[:, sl])

            NH = OUT_W // 2
            p2a = ps2.tile([SLOTS_PER_GROUP, NH], F32, space="PSUM", tag="p2a")
            p2b = ps2.tile([SLOTS_PER_GROUP, NH], F32, space="PSUM", tag="p2b")
            nc.tensor.matmul(p2a[:], lhsT=hf[:], rhs=w2_sb[:, :NH],
                             start=True, stop=True)
            nc.tensor.matmul(p2b[:], lhsT=hf[:], rhs=w2_sb[:, NH:],
                             start=True, stop=True)
            o_sb = hpool.tile([SLOTS_PER_GROUP, OUT_W], F32, tag="o")
            nc.vector.tensor_copy(out=o_sb[:, :NH], in_=p2a[:])
            nc.vector.tensor_copy(out=o_sb[:, NH:], in_=p2b[:])
            nc.sync.dma_start(out=out[sl, :], in_=o_sb[:])

    nc.compile()
    return nc


def _host_prep_mlp(leaves, x, w1s, b1s, w2s):
    in_maps, slot_maps = [], []
    order = np.argsort(leaves, kind="stable")
    sorted_leaves = leaves[order]
    for c in range(N_CORES):
        lo, hi = LEAVES_PER_CORE * c, LEAVES_PER_CORE * (c + 1)
        beg, end = np.searchsorted(sorted_leaves, [lo, hi])
        samples = order[beg:end]
        l_loc = leaves[samples] - lo
        g_all = l_loc // EXPERTS_PER_GROUP
        e_all = l_loc % EXPERTS_PER_GROUP
        slot = np.empty(len(samples), np.int64)
        fill = np.zeros(GROUPS, np.int64)
        for i, g in enumerate(g_all):
            slot[i] = SLOTS_PER_GROUP * g + fill[g]
            fill[g] += 1
        if len(fill) and fill.max() > SLOTS_PER_GROUP:
            raise RuntimeError(f"group capacity exceeded: {fill.max()}")

        slot_sample = np.full(SLOTS, -1, np.int64)
        slot_sample[slot] = samples
        mask = np.zeros((128, SLOTS), np.float32)
        lane_rows = (16 * e_all[None, :] + np.arange(16)[:, None])
        mask[lane_rows, slot[None, :]] = 1.0

        xg = np.zeros((SLOTS, IN_W), np.float32)
        xg[slot] = x[samples]
        xgT = np.ascontiguousarray(xg.T)

        w1f = np.ascontiguousarray(
            w1s[lo:hi].reshape(GROUPS, 8, IN_W, LEAF_W)
            .transpose(0, 2, 1, 3)
            .reshape(GROUPS, IN_W, 128)
            .reshape(GROUPS, KC, 128, 128)
            .transpose(0, 2, 1, 3)
            .reshape(GROUPS, 128, KC * 128)
        )
        w2f = np.ascontiguousarray(w2s[lo:hi].reshape(GROUPS, 128, OUT_W))
        b1bc = np.ascontiguousarray(
            b1s[lo:hi].reshape(GROUPS, 128).T
        ).astype(np.float32)

        in_maps.append({"xgT": xgT, "w1f": w1f, "w2f": w2f,
                        "b1bc": b1bc, "maskt": mask})
        slot_maps.append(slot_sample)
    return in_maps, slot_maps


# ---------------------------------------------------------------- entry
def kernel(x, node_weights, node_biases, w1s, b1s, w2s):
    x = np.ascontiguousarray(np.asarray(x, np.float32))
    node_weights = np.ascontiguousarray(np.asarray(node_weights, np.float32))
    node_biases = np.ascontiguousarray(np.asarray(node_biases, np.float32))
    w1s = np.asarray(w1s, np.float32)
    b1s = np.asarray(b1s, np.float32)
    w2s = np.asarray(w2s, np.float32)

    # launch 1: routing
    nc1 = _build_routing_nc()
    in1 = _host_prep_routing(x, node_weights, node_biases)
    res1 = run_bass_kernel_spmd(nc1, in1, core_ids=list(range(N_CORES)))
    leaves = np.concatenate([res1.results[c]["leaf"] for c in range(N_CORES)])
    leaves = leaves.astype(np.int64)

    # launch 2: expert MLP
    nc2 = _build_mlp_nc()
    in2, slot_maps = _host_prep_mlp(leaves, x, w1s, b1s, w2s)
    res2 = run_bass_kernel_spmd(nc2, in2, core_ids=list(range(N_CORES)))

    out = np.zeros((BATCH, OUT_W), np.float32)
    for c in range(N_CORES):
        o_slots = res2.results[c]["o"]
        sm = slot_maps[c]
        valid = sm >= 0
        out[sm[valid]] = o_slots[valid]
    return out
